# revision 5
# baseline (speedup 1.0000x reference)
"""KimiMoEGate on 8 Trainium2 NeuronCores.

Data-parallel over tokens: each core takes 1024 tokens, the full gate weight,
and produces (topk_idx, topk_weight) for its shard.

Single-pass fp32r GEMM: the PE runs float32r matmuls at full fp16 row rate
when the output free dim is >= 256, so logits are computed exactly in one
pass (no hi/lo split, no ACT/DVE split traffic).  The kernel is then
DMA-bound: 29.4 MB of x + 7.3 MB of w per core at ~360 GB/s ~= 102 us.

Routing epilogue per 128-token block: sigmoid on ACT; grouped top-2 /
top-4-group mask / top-8 experts via DVE max8/max_index/match_replace;
weights recovered order-exactly via an 8x8 equality match.
"""
import os
import sys
sys.path.insert(0, '/opt/trn_rl_repo')
import numpy as np
import concourse.bass as bass
from concourse import bacc
import concourse.mybir as mybir
from concourse.bass_utils import run_bass_kernel_spmd
from concourse.tile import TileContext

F32 = mybir.dt.float32
F32R = mybir.dt.float32r
U32 = mybir.dt.uint32
I32 = mybir.dt.int32
AX = mybir.AxisListType
ALU = mybir.AluOpType
ACTF = mybir.ActivationFunctionType

T, H, E = 8192, 7168, 256
NCORES = 8
TPC = T // NCORES            # 1024 tokens per core
KT = H // 128                # 56 contraction tiles
NB = TPC // 128              # 8 blocks of 128 tokens
NEG = -1e30

_cache = {}
LAST = None                  # BassKernelResults of the most recent run


def _build(repeat=1):
    key = ("nc", repeat)
    if key in _cache:
        return _cache[key]
    nc = bacc.Bacc("TRN2", target_bir_lowering=False, debug=False,
                   num_devices=NCORES)
    xall = nc.dram_tensor("xall", [128, NB, KT, 128], F32, kind="ExternalInput")
    wall = nc.dram_tensor("wall", [128, KT, E], F32, kind="ExternalInput")
    bias = nc.dram_tensor("bias", [E], F32, kind="ExternalInput")
    o_idx = nc.dram_tensor("o_idx", [TPC, 8], I32, kind="ExternalOutput")
    o_w = nc.dram_tensor("o_w", [TPC, 8], F32, kind="ExternalOutput")

    with TileContext(nc) as tc:
        with (
            tc.tile_pool(name="wpool", bufs=1) as wpool,
            tc.tile_pool(name="xpool", bufs=3) as xpool,
            tc.tile_pool(name="small", bufs=2) as small,
            tc.tile_pool(name="ps", bufs=2, space="PSUM") as ps,
        ):
            wsb = wpool.tile([128, KT, E], F32)
            nc.sync.dma_start(wsb[:], wall[:])
            bias_rep = wpool.tile([128, E], F32)
            nc.sync.dma_start(bias_rep[:], bias[None, :].to_broadcast([128, E]))

            for b in range(NB * repeat):
                b = b % NB
                xsb = xpool.tile([128, KT, 128], F32, tag="x")
                nc.sync.dma_start(xsb[:], xall[:, b])

                psl = ps.tile([128, E], F32, tag="ps")
                for k in range(KT):
                    nc.tensor.matmul(psl[:], xsb[:, k].bitcast(F32R),
                                     wsb[:, k].bitcast(F32R),
                                     start=(k == 0), stop=(k == KT - 1))

                # ---- epilogue: sigmoid, grouped top-k routing ----
                s = small.tile([128, E], F32, tag="s")
                nc.scalar.activation(s[:], psl[:], ACTF.Sigmoid)
                sc = small.tile([128, E], F32, tag="sc")
                nc.vector.tensor_tensor(sc[:], s[:], bias_rep[:], ALU.add)

                scg = sc[:].rearrange("p (g e) -> p g e", g=8)
                gm = small.tile([128, 8], F32, tag="gm")
                nc.vector.tensor_reduce(gm[:], scg, AX.X, ALU.max)
                scr = small.tile([128, E], F32, tag="scr")
                nc.vector.match_replace(scr[:], gm[:], sc[:], NEG)
                gm2 = small.tile([128, 8], F32, tag="gm2")
                nc.vector.tensor_reduce(
                    gm2[:], scr[:].rearrange("p (g e) -> p g e", g=8),
                    AX.X, ALU.max)
                gsum = small.tile([128, 8], F32, tag="gsum")
                nc.vector.tensor_tensor(gsum[:], gm[:], gm2[:], ALU.add)
                g8 = small.tile([128, 8], F32, tag="g8")
                nc.vector.max(g8[:], gsum[:])
                gmask = small.tile([128, 8], F32, tag="gmask")
                nc.vector.tensor_scalar(gmask[:], gsum[:], g8[:, 3:4], None,
                                        op0=ALU.is_ge)
                tmp = small.tile([128, E], F32, tag="tmp")
                nc.vector.tensor_tensor(
                    tmp[:].rearrange("p (g e) -> p g e", g=8), scg,
                    gmask[:, :, None].to_broadcast([128, 8, 32]), ALU.mult)
                v8 = small.tile([128, 8], F32, tag="v8")
                nc.vector.max(v8[:], tmp[:])
                i8 = small.tile([128, 8], U32, tag="i8")
                nc.vector.max_index(i8[:], v8[:], tmp[:])

                marked = small.tile([128, E], F32, tag="marked")
                nc.vector.match_replace(marked[:], v8[:], tmp[:], NEG)
                possel = small.tile([128, E], F32, tag="possel")
                nc.vector.tensor_tensor(possel[:], tmp[:], marked[:],
                                        ALU.not_equal)
                s_sel = small.tile([128, E], F32, tag="s_sel")
                nc.vector.tensor_tensor(s_sel[:], s[:], possel[:], ALU.mult)
                w8s = small.tile([128, 8], F32, tag="w8s")
                nc.vector.max(w8s[:], s_sel[:])
                is8 = small.tile([128, 8], U32, tag="is8")
                nc.vector.max_index(is8[:], w8s[:], s_sel[:])

                eq = small.tile([128, 8, 8], F32, tag="eq")
                nc.vector.tensor_tensor(
                    eq[:],
                    is8[:, None, :].to_broadcast([128, 8, 8]),
                    i8[:, :, None].to_broadcast([128, 8, 8]),
                    ALU.is_equal)
                prod = small.tile([128, 8, 8], F32, tag="prod")
                nc.vector.tensor_tensor(
                    prod[:], eq[:],
                    w8s[:, None, :].to_broadcast([128, 8, 8]), ALU.mult)
                w8 = small.tile([128, 8], F32, tag="w8")
                nc.vector.tensor_reduce(w8[:], prod[:], AX.X, ALU.add)
                ssum = small.tile([128, 1], F32, tag="ssum")
                nc.vector.tensor_reduce(ssum[:], w8s[:], AX.X, ALU.add)
                rec = small.tile([128, 1], F32, tag="rec")
                nc.vector.reciprocal(rec[:], ssum[:])
                rec25 = small.tile([128, 1], F32, tag="rec25")
                nc.vector.tensor_scalar(rec25[:], rec[:], 2.5, None,
                                        op0=ALU.mult)
                wfin = small.tile([128, 8], F32, tag="wfin")
                nc.vector.tensor_scalar(wfin[:], w8[:], rec25[:], None,
                                        op0=ALU.mult)
                nc.sync.dma_start(o_w[b * 128:(b + 1) * 128], wfin[:])
                nc.sync.dma_start(o_idx[b * 128:(b + 1) * 128],
                                  i8[:].bitcast(I32))
    nc.compile()
    _cache[key] = nc
    return nc


def prep_in_maps(hidden_states, weight, e_score_correction_bias):
    x = np.asarray(hidden_states, dtype=np.float32)
    w = np.asarray(weight, dtype=np.float32)
    b = np.asarray(e_score_correction_bias, dtype=np.float32)

    # layout prep (host): transposed so the contraction dim lands on
    # partitions, contiguous per (partition, block)
    wall = np.ascontiguousarray(w.T.reshape(KT, 128, E).transpose(1, 0, 2))
    in_maps = []
    for c in range(NCORES):
        xs = x[c * TPC:(c + 1) * TPC]                     # [1024, H]
        xt = xs.T.reshape(KT, 128, NB, 128)               # [k, p, b, n]
        xc = np.ascontiguousarray(xt.transpose(1, 2, 0, 3))  # [p, b, k, n]
        in_maps.append({"xall": xc, "wall": wall, "bias": b})
    return in_maps


def kernel(hidden_states, weight, e_score_correction_bias):
    global LAST
    nc = _build()
    in_maps = prep_in_maps(hidden_states, weight, e_score_correction_bias)
    res = run_bass_kernel_spmd(nc, in_maps, list(range(NCORES)))
    LAST = res
    r = res.results
    idx = np.concatenate([r[c]["o_idx"] for c in range(NCORES)], axis=0)
    wgt = np.concatenate([r[c]["o_w"] for c in range(NCORES)], axis=0)
    return idx.astype(np.int32), wgt.astype(np.float32)
